# revision 6
# baseline (speedup 1.0000x reference)
"""Multi-head GAT layer (4 heads, mean-aggregated) + residual + GraphNorm + gelu
on 8 Trainium2 NeuronCores (SPMD, one NEFF on all cores).

Strategy:
  - dst nodes partitioned contiguously across the 8 cores (12500 each); every
    edge is processed by the core owning its destination.
  - Every core runs one fused matmul sweep over ALL nodes producing per-node
    records [xl(256) | a_src(4) | pad] into a DRAM gather table (4 chunk
    tensors of 25088 rows so edge gathers can start before the whole table is
    written). A second tiny sweep over the core's OWN nodes produces a_dst and
    the residual.
  - Edge phase: per 128-dst tile, dma_gather (4 SWDGE queues, int16 chunk-local
    indices) pulls the per-edge src records; softmax over incoming edges is
    computed without the max shift (same result; alpha is bounded here); the
    segmented sum is a one-hot matmul into PSUM with the softmax denominator
    folded in as 4 extra columns.
  - GraphNorm: per-graph sums of h and h^2 via one-hot matmuls, AllReduced
    across cores, then per-node affine + gelu.

The edge structure (group sizes per tile/chunk) is made identical across cores
by padding each group to the max over cores, so one SPMD NEFF serves all 8.
"""
import numpy as np

N, F, C, H, E, B = 100000, 128, 64, 4, 1200000, 8
NCORE = 8
NEG = 0.2
EPS = 1e-5
NOWN = N // NCORE             # 12500 dst nodes per core
TILES = (NOWN + 127) // 128   # 98 dst tiles per core (last partial: 84 rows)
LAST_ROWS = NOWN - (TILES - 1) * 128
NT = (N + 127) // 128         # 782 node tiles
NPAD = NT * 128               # 100096
NCH = 4
CHR = 25088                   # chunk rows; NCH*CHR >= NPAD
NTC = CHR // 128              # 196 node tiles per chunk
REC = 320                     # record elems: [xl 256 | asrc 4 | junk 60] (1280B)

_F32 = np.float32


def _cdiv(a, b):
    return (a + b - 1) // b


def _host_prep(x, edge_index, batch, W, att_src, att_dst, bias_gat, res_W,
               res_b, gn_weight, gn_bias, gn_mean_scale):
    """Compute static structure + per-core input tensors."""
    x = np.asarray(x, _F32)
    W = np.asarray(W, _F32)
    att_src = np.asarray(att_src, _F32)
    att_dst = np.asarray(att_dst, _F32)
    res_W = np.asarray(res_W, _F32)
    batch = np.asarray(batch).astype(np.int64)

    # fused node-sweep right matrix [F, 328] = [W.T | As.T | Ad.T | res_W.T]
    W3 = W.reshape(H, C, F)
    As = (att_src[:, :, None] * W3).sum(1)          # [H, F]
    Ad = (att_dst[:, :, None] * W3).sum(1)          # [H, F]
    Rcat = np.concatenate([W.T, As.T, Ad.T, res_W.T], axis=1).astype(_F32)

    xT = np.zeros((F, NPAD), _F32)
    xT[:, :N] = x.T

    # ---- edges (+ self loops), assigned to cores by dst ----
    loop = np.arange(N, dtype=np.int64)
    src = np.concatenate([np.asarray(edge_index[0]), loop]).astype(np.int64)
    dst = np.concatenate([np.asarray(edge_index[1]), loop]).astype(np.int64)
    owner = dst // NOWN
    tl = (dst % NOWN) // 128
    dl = (dst % NOWN) % 128
    ch = src // CHR

    key = (owner * TILES + tl) * NCH + ch
    counts = np.bincount(key, minlength=NCORE * TILES * NCH).reshape(
        NCORE, TILES, NCH)
    K_tc = counts.max(axis=0).astype(np.int64)       # [TILES, NCH]
    nb_tc = _cdiv(K_tc, 128)                         # gather blocks per group
    n_sub = nb_tc.sum(axis=1)                        # [TILES]
    ic_tc = _cdiv(K_tc, 16)                          # idx cols per group

    B0 = np.zeros((TILES, NCH), np.int64)            # block offset within tile
    B0[:, 1:] = np.cumsum(nb_tc, axis=1)[:, :-1]
    jb = np.zeros(TILES + 1, np.int64)               # dstloc col offset per tile
    jb[1:] = np.cumsum(n_sub)
    NSUBTOT = int(jb[-1])
    O = np.zeros(TILES * NCH, np.int64)              # idx16 col offsets
    O_flat = np.cumsum(ic_tc.flatten())
    O[1:] = O_flat[:-1]
    O = O.reshape(TILES, NCH)
    IDXC = int(O_flat[-1])

    order = np.lexsort((ch, tl, owner))
    s_src, s_dl = src[order], dl[order]
    gstart = np.zeros(NCORE * TILES * NCH + 1, np.int64)
    gstart[1:] = np.cumsum(counts.flatten())

    in_maps = []
    for k in range(NCORE):
        idx16 = np.zeros((128, IDXC), np.int16)
        dstloc = np.full((128, NSUBTOT), -1.0, _F32)
        for t in range(TILES):
            for c in range(NCH):
                K = int(K_tc[t, c])
                if K == 0:
                    continue
                gi = (k * TILES + t) * NCH + c
                n = int(counts[k, t, c])
                a = int(gstart[gi])
                loc = (s_src[a:a + n] - c * CHR).astype(np.int16)
                padded = np.zeros(_cdiv(K, 16) * 16, np.int16)
                padded[:n] = loc
                blk = padded.reshape(-1, 16).T       # [16, icols]
                cols = blk.shape[1]
                idx16[:, O[t, c]:O[t, c] + cols] = np.tile(blk, (8, 1))
                dlv = np.full(int(nb_tc[t, c]) * 128, -1.0, _F32)
                dlv[:n] = s_dl[a:a + n].astype(_F32)
                j0 = int(jb[t] + B0[t, c])
                dstloc[:, j0:j0 + int(nb_tc[t, c])] = dlv.reshape(-1, 128).T

        base = k * NOWN
        xTo = np.zeros((F, TILES * 128), _F32)
        xTo[:, :NOWN] = x[base:base + NOWN].T
        bslice = batch[base:base + NOWN]
        onehot_b = np.zeros((128, TILES * 8), _F32)
        onehotT = np.zeros((8, TILES * 128), _F32)
        node_ids = np.arange(NOWN)
        pp = node_ids % 128
        tt = node_ids // 128
        onehot_b[pp, tt * 8 + bslice] = 1.0
        onehotT[bslice, node_ids] = 1.0

        in_maps.append({
            "xT": xT, "xTo": xTo, "Rcat": Rcat, "idx16": idx16,
            "dstloc": dstloc, "onehot_b": onehot_b, "onehotT": onehotT,
        })

    bc_row = np.tile((np.asarray(bias_gat, _F32)
                      + np.asarray(res_b, _F32))[None, :], (128, 1))
    iota_row = np.tile(np.arange(128, dtype=_F32)[None, :], (128, 1))
    iota_part = np.arange(128, dtype=_F32)[:, None].copy()
    ident = np.eye(128, dtype=_F32)
    alpha_t = np.full((128, 1), NEG, _F32)
    gms = np.asarray(gn_mean_scale, _F32)
    cnt = np.bincount(batch, minlength=B).astype(_F32)
    gn_pack = np.zeros((8, 4 * C + 2), _F32)
    gn_pack[:, 0:C] = np.asarray(gn_weight, _F32)[None, :]
    gn_pack[:, C:2 * C] = np.asarray(gn_bias, _F32)[None, :]
    gn_pack[:, 2 * C:3 * C] = gms[None, :]
    gn_pack[:, 3 * C:4 * C] = (gms * (2.0 - gms))[None, :]
    gn_pack[:, 4 * C] = 1.0 / cnt
    gn_pack[:, 4 * C + 1] = EPS
    for m in in_maps:
        m.update({"bc_row": bc_row, "iota_row": iota_row,
                  "iota_part": iota_part, "ident": ident,
                  "alpha_t": alpha_t, "gn_pack": gn_pack})

    cfg = {
        "K_tc": K_tc, "nb_tc": nb_tc, "n_sub": n_sub, "B0": B0, "jb": jb,
        "O": O, "ic_tc": ic_tc, "NSUBTOT": NSUBTOT, "IDXC": IDXC,
        "MAXSUB": int(n_sub.max()),
    }
    return cfg, in_maps


def _build_nc(cfg):
    import concourse.bacc as bacc
    import concourse.mybir as mybir
    import concourse.tile as tile

    AF = mybir.ActivationFunctionType
    OP = mybir.AluOpType
    f32 = mybir.dt.float32
    i16 = mybir.dt.int16

    K_tc, nb_tc, n_sub = cfg["K_tc"], cfg["nb_tc"], cfg["n_sub"]
    B0, jb, O = cfg["B0"], cfg["jb"], cfg["O"]
    NSUBTOT, IDXC, MAXSUB = cfg["NSUBTOT"], cfg["IDXC"], cfg["MAXSUB"]

    nc = bacc.Bacc("TRN2", target_bir_lowering=False, num_swdge_queues=4)

    xT = nc.declare_dram_parameter("xT", [F, NPAD], f32, isOutput=False)
    xTo = nc.declare_dram_parameter("xTo", [F, TILES * 128], f32, isOutput=False)
    Rcat = nc.declare_dram_parameter("Rcat", [F, 328], f32, isOutput=False)
    idx16 = nc.declare_dram_parameter("idx16", [128, IDXC], i16, isOutput=False)
    dstloc = nc.declare_dram_parameter("dstloc", [128, NSUBTOT], f32, isOutput=False)
    onehot_b = nc.declare_dram_parameter("onehot_b", [128, TILES * 8], f32, isOutput=False)
    onehotT = nc.declare_dram_parameter("onehotT", [8, TILES * 128], f32, isOutput=False)
    bc_row = nc.declare_dram_parameter("bc_row", [128, C], f32, isOutput=False)
    iota_row = nc.declare_dram_parameter("iota_row", [128, 128], f32, isOutput=False)
    iota_part = nc.declare_dram_parameter("iota_part", [128, 1], f32, isOutput=False)
    ident = nc.declare_dram_parameter("ident", [128, 128], f32, isOutput=False)
    alpha_t = nc.declare_dram_parameter("alpha_t", [128, 1], f32, isOutput=False)
    gn_pack = nc.declare_dram_parameter("gn_pack", [8, 4 * C + 2], f32, isOutput=False)
    out = nc.declare_dram_parameter("out", [NOWN, C], f32, isOutput=True)

    tables = [nc.dram_tensor(f"table{c}", [CHR, REC], f32) for c in range(NCH)]
    cc_in = nc.dram_tensor("cc_in", [8, 2 * C], f32)
    cc_out = nc.dram_tensor("cc_out", [8, 2 * C], f32)

    GRP = 7          # node tiles per phase-1 write group (196 = 28*7)
    NGRP = NTC // GRP
    qn = [0]

    def next_q():
        q = qn[0]
        qn[0] = (q + 1) % 4
        return q

    with tile.TileContext(nc) as tc:
        with (
            tc.tile_pool(name="const", bufs=1) as cp,
            tc.tile_pool(name="persist", bufs=1) as pers,
            tc.tile_pool(name="xload", bufs=2) as xp,
            tc.tile_pool(name="recw", bufs=3) as rp,
            tc.tile_pool(name="gat", bufs=2) as gp,
            tc.tile_pool(name="m1", bufs=2) as m1p,
            tc.tile_pool(name="m2", bufs=3) as m2p,
            tc.tile_pool(name="rhs", bufs=4) as rhp,
            tc.tile_pool(name="small", bufs=4) as smp,
            tc.tile_pool(name="idxp", bufs=2) as ixp,
        ):
            # ---- constants into SBUF ----
            rc_sb = cp.tile([F, 328], f32)
            nc.sync.dma_start(rc_sb[:], Rcat[:])
            iota_sb = cp.tile([128, 128], f32)
            nc.sync.dma_start(iota_sb[:], iota_row[:])
            iotap_sb = cp.tile([128, 1], f32)
            nc.sync.dma_start(iotap_sb[:], iota_part[:])
            id_sb = cp.tile([128, 128], f32)
            nc.sync.dma_start(id_sb[:], ident[:])
            bc_sb = cp.tile([128, C], f32)
            nc.sync.dma_start(bc_sb[:], bc_row[:])
            ohb_sb = cp.tile([128, TILES * 8], f32)
            nc.sync.dma_start(ohb_sb[:], onehot_b[:])
            dl_sb = cp.tile([128, NSUBTOT], f32)
            nc.sync.dma_start(dl_sb[:], dstloc[:])
            al_sb = cp.tile([128, 1], f32)
            nc.sync.dma_start(al_sb[:], alpha_t[:])
            gn_sb = cp.tile([8, 4 * C + 2], f32)
            nc.sync.dma_start(gn_sb[:], gn_pack[:])

            adst_sb = pers.tile([128, TILES * 4], f32)
            resid_sb = pers.tile([128, TILES * C], f32)
            h_sb = pers.tile([128, TILES * C], f32)
            stats_sb = pers.tile([8, 2 * C], f32)
            nc.vector.memset(stats_sb[:], 0.0)

            with tc.tile_pool(name="psum1", bufs=3, space="PSUM") as ps1:
                # ---- phase 1b: owned-node sweep -> a_dst + residual ----
                for t in range(TILES):
                    xs = xp.tile([F, 128], f32, tag="xo")
                    nc.sync.dma_start(xs[:], xTo[:, t * 128:(t + 1) * 128])
                    ps = ps1.tile([128, 68], f32, tag="ops")
                    nc.tensor.matmul(ps[:], lhsT=xs[:], rhs=rc_sb[:, 260:328],
                                     start=True, stop=True)
                    nc.vector.tensor_copy(adst_sb[:, t * 4:(t + 1) * 4],
                                          ps[:, 0:4])
                    nc.vector.tensor_tensor(
                        out=resid_sb[:, t * C:(t + 1) * C],
                        in0=ps[:, 4:68], in1=bc_sb[:], op=OP.add)

                # ---- phase 1a: record table build (all nodes) ----
                for c in range(NCH):
                    ntc_real = min(NTC, NT - c * NTC)
                    for g0 in range(0, ntc_real, GRP):
                        ng = min(GRP, ntc_real - g0)
                        t0 = c * NTC + g0
                        xs = xp.tile([F, GRP * 128], f32, tag="x")
                        nc.sync.dma_start(
                            xs[:, 0:ng * 128], xT[:, t0 * 128:(t0 + ng) * 128])
                        rec = rp.tile([128, GRP * REC], f32, tag="rec")
                        for i in range(ng):
                            ps = ps1.tile([128, 328], f32, tag="nps")
                            nc.tensor.matmul(
                                ps[:], lhsT=xs[:, i * 128:(i + 1) * 128],
                                rhs=rc_sb[:], start=True, stop=True)
                            if i % 2 == 0:
                                nc.vector.tensor_copy(
                                    rec[:, i * REC:(i + 1) * REC], ps[:, 0:REC])
                            else:
                                nc.scalar.copy(
                                    rec[:, i * REC:(i + 1) * REC], ps[:, 0:REC])
                        nc.sync.dma_start(
                            tables[c][g0 * 128:(g0 + ng) * 128, :]
                            .rearrange("(i p) e -> p i e", p=128),
                            rec[:, 0:ng * REC].rearrange("p (i e) -> p i e", e=REC))

            # ---- phase 2: edge sweep over owned dst tiles ----
            with (
                tc.tile_pool(name="psum_dlt", bufs=2, space="PSUM") as psd,
                tc.tile_pool(name="psum_adst", bufs=2, space="PSUM") as psa,
                tc.tile_pool(name="psum_agg", bufs=2, space="PSUM") as psg,
                tc.tile_pool(name="psum_stat", bufs=1, space="PSUM") as pss,
            ):
                for t in range(TILES):
                    ns = int(n_sub[t])
                    J = int(jb[t])
                    g = gp.tile([128, MAXSUB * REC], f32, tag="g")
                    if t < 2:
                        # prime the pool slots: later tiles inherit old
                        # (finite) gather data in any region they don't write,
                        # but virgin SBUF may hold NaN bit patterns.
                        nc.vector.memset(g[:], 0.0)
                    for c in range(NCH):
                        K = int(K_tc[t, c])
                        if K == 0:
                            continue
                        nb = int(nb_tc[t, c])
                        b0 = int(B0[t, c])
                        o0 = int(O[t, c])
                        oc = _cdiv(K, 16)
                        ix = ixp.tile([128, _cdiv(K, 16)], i16, tag="ix")
                        nc.sync.dma_start(ix[:], idx16[:, o0:o0 + oc])
                        nc.gpsimd.dma_gather(
                            out_ap=g[:, b0 * REC:(b0 + nb) * REC]
                            .rearrange("p (j e) -> p j e", e=REC),
                            in_ap=tables[c][:],
                            idxs_ap=ix[:],
                            num_idxs=K, num_idxs_reg=K,
                            elem_size=REC, queue_num=next_q())
                    # batched M1: [128, ns*128] one-hot (edge-partitioned)
                    m1 = m1p.tile([128, MAXSUB * 128], f32, tag="m1")
                    nc.vector.tensor_tensor(
                        out=m1[:, 0:ns * 128].rearrange("p (j i) -> p j i", i=128),
                        in0=dl_sb[:, J:J + ns].to_broadcast([128, ns, 128]),
                        in1=iota_sb[:].unsqueeze(1).to_broadcast([128, ns, 128]),
                        op=OP.is_equal)
                    # a_dst broadcast to edges via transposed one-hot matmuls
                    ape = psa.tile([128, MAXSUB * 4], f32, tag="ape")
                    for j in range(ns):
                        dlt = psd.tile([128, 128], f32, tag="dlt")
                        nc.tensor.transpose(
                            dlt[:], dl_sb[:, J + j:J + j + 1].to_broadcast([128, 128]),
                            id_sb[:])
                        m2 = m2p.tile([128, 128], f32, tag="m2")
                        nc.vector.tensor_tensor(
                            out=m2[:], in0=iotap_sb[:, 0:1].to_broadcast([128, 128]),
                            in1=dlt[:], op=OP.is_equal)
                        nc.tensor.matmul(
                            ape[:, j * 4:(j + 1) * 4], lhsT=m2[:],
                            rhs=adst_sb[:, t * 4:(t + 1) * 4],
                            start=True, stop=True)
                    # alpha = asrc[src] + adst[dst]; ex = exp(leakyrelu(alpha))
                    lr = smp.tile([128, MAXSUB * 4], f32, tag="lr")
                    nc.vector.tensor_tensor(
                        out=lr[:, 0:ns * 4].rearrange("p (j h) -> p j h", h=4),
                        in0=g[:].rearrange("p (j e) -> p j e", e=REC)[:, 0:ns, 256:260],
                        in1=ape[:, 0:ns * 4].rearrange("p (j h) -> p j h", h=4),
                        op=OP.add)
                    ex = smp.tile([128, MAXSUB * 4], f32, tag="ex")
                    nc.scalar.activation(out=ex[:, 0:ns * 4], in_=lr[:, 0:ns * 4],
                                         func=AF.Prelu, alpha=al_sb[:, 0:1])
                    nc.scalar.activation(out=ex[:, 0:ns * 4], in_=ex[:, 0:ns * 4],
                                         func=AF.Exp)
                    # weighted segsum via one-hot matmul, denom in cols 256:260
                    agg = psg.tile([128, 260], f32, tag="agg")
                    for j in range(ns):
                        rhs = rhp.tile([128, 260], f32, tag="rhs")
                        nc.vector.tensor_tensor(
                            out=rhs[:, 0:256].rearrange("p (h c) -> p h c", c=C),
                            in0=g[:, j * REC:j * REC + 256]
                            .rearrange("p (h c) -> p h c", c=C),
                            in1=ex[:, j * 4:(j + 1) * 4].to_broadcast([128, 4, C]),
                            op=OP.mult)
                        nc.scalar.copy(rhs[:, 256:260], ex[:, j * 4:(j + 1) * 4])
                        nc.tensor.matmul(
                            agg[:], lhsT=m1[:, j * 128:(j + 1) * 128], rhs=rhs[:],
                            start=(j == 0), stop=(j == ns - 1))
                    # combine heads: h = 0.25*sum_h agg_h/denom_h + resid(+bias)
                    dn = smp.tile([128, 4], f32, tag="dn")
                    nc.vector.tensor_scalar(
                        out=dn[:], in0=agg[:, 256:260], scalar1=1e-6,
                        scalar2=None, op0=OP.add)
                    recip = smp.tile([128, 4], f32, tag="recip")
                    nc.vector.reciprocal(recip[:], dn[:])
                    hacc = smp.tile([128, C], f32, tag="hacc")
                    nc.vector.tensor_scalar(
                        out=hacc[:], in0=agg[:, 0:C], scalar1=recip[:, 0:1],
                        scalar2=None, op0=OP.mult)
                    for h in range(1, H):
                        nc.vector.scalar_tensor_tensor(
                            out=hacc[:], in0=agg[:, h * C:(h + 1) * C],
                            scalar=recip[:, h:h + 1], in1=hacc[:],
                            op0=OP.mult, op1=OP.add)
                    hsl = h_sb[:, t * C:(t + 1) * C]
                    nc.vector.scalar_tensor_tensor(
                        out=hsl, in0=hacc[:], scalar=1.0 / H,
                        in1=resid_sb[:, t * C:(t + 1) * C],
                        op0=OP.mult, op1=OP.add)
                    # graphnorm partial stats
                    sq = smp.tile([128, C], f32, tag="sq")
                    nc.vector.tensor_tensor(out=sq[:], in0=hsl, in1=hsl, op=OP.mult)
                    st = pss.tile([8, 2 * C], f32, tag="st")
                    nc.tensor.matmul(st[:, 0:C], lhsT=ohb_sb[:, t * 8:(t + 1) * 8],
                                     rhs=hsl, start=True, stop=True)
                    nc.tensor.matmul(st[:, C:2 * C], lhsT=ohb_sb[:, t * 8:(t + 1) * 8],
                                     rhs=sq[:], start=True, stop=True)
                    nc.vector.tensor_tensor(out=stats_sb[:], in0=stats_sb[:],
                                            in1=st[:], op=OP.add)

            # ---- phase 3: AllReduce stats, normalize, gelu, write out ----
            with tc.tile_pool(name="psum3", bufs=2, space="PSUM") as ps3:
                nc.gpsimd.dma_start(cc_in[:], stats_sb[:])
                nc.gpsimd.collective_compute(
                    "AllReduce", OP.add,
                    replica_groups=[list(range(NCORE))],
                    ins=[cc_in[:]], outs=[cc_out[:]])
                sall = smp.tile([8, 2 * C], f32, tag="sall")
                nc.sync.dma_start(sall[:], cc_out[:])
                gw = gn_sb[:, 0:C]
                gb = gn_sb[:, C:2 * C]
                gms = gn_sb[:, 2 * C:3 * C]
                gms2m = gn_sb[:, 3 * C:4 * C]
                invc = gn_sb[:, 4 * C:4 * C + 1]
                epsc = gn_sb[:, 4 * C + 1:4 * C + 2]
                mean = smp.tile([8, C], f32, tag="mean")
                nc.vector.tensor_scalar(out=mean[:], in0=sall[:, 0:C],
                                        scalar1=invc, scalar2=None, op0=OP.mult)
                eh2 = smp.tile([8, C], f32, tag="eh2")
                nc.vector.tensor_scalar(out=eh2[:], in0=sall[:, C:2 * C],
                                        scalar1=invc, scalar2=None, op0=OP.mult)
                msq = smp.tile([8, C], f32, tag="msq")
                nc.vector.tensor_tensor(out=msq[:], in0=mean[:], in1=mean[:],
                                        op=OP.mult)
                var = smp.tile([8, C], f32, tag="var")
                # var = eh2 - msq*gms2m
                nc.vector.tensor_tensor(out=msq[:], in0=msq[:], in1=gms2m,
                                        op=OP.mult)
                nc.vector.tensor_tensor(out=var[:], in0=eh2[:], in1=msq[:],
                                        op=OP.subtract)
                std = smp.tile([8, C], f32, tag="std")
                nc.scalar.activation(out=std[:], in_=var[:], func=AF.Sqrt,
                                     bias=epsc)
                ab = smp.tile([8, 2 * C], f32, tag="ab")
                nc.vector.reciprocal(std[:], std[:])
                # A = gw/std ; B = gb - A*mean*gms
                nc.vector.tensor_tensor(out=ab[:, 0:C], in0=gw, in1=std[:],
                                        op=OP.mult)
                tm = smp.tile([8, C], f32, tag="tm")
                nc.vector.tensor_tensor(out=tm[:], in0=ab[:, 0:C], in1=mean[:],
                                        op=OP.mult)
                nc.vector.tensor_tensor(out=tm[:], in0=tm[:], in1=gms,
                                        op=OP.mult)
                nc.vector.tensor_tensor(out=ab[:, C:2 * C], in0=gb, in1=tm[:],
                                        op=OP.subtract)

                for t in range(TILES):
                    oht = ixp.tile([8, 128], f32, tag="oht")
                    nc.sync.dma_start(oht[:], onehotT[:, t * 128:(t + 1) * 128])
                    abpe = ps3.tile([128, 2 * C], f32, tag="abpe")
                    nc.tensor.matmul(abpe[:], lhsT=oht[:], rhs=ab[:],
                                     start=True, stop=True)
                    nrm = smp.tile([128, C], f32, tag="nrm")
                    nc.vector.tensor_tensor(out=nrm[:],
                                            in0=h_sb[:, t * C:(t + 1) * C],
                                            in1=abpe[:, 0:C], op=OP.mult)
                    nc.vector.tensor_tensor(out=nrm[:], in0=nrm[:],
                                            in1=abpe[:, C:2 * C], op=OP.add)
                    ot = smp.tile([128, C], f32, tag="ot")
                    nc.scalar.activation(out=ot[:], in_=nrm[:],
                                         func=AF.Gelu_apprx_tanh)
                    rows = 128 if t < TILES - 1 else LAST_ROWS
                    nc.sync.dma_start(out[t * 128:t * 128 + rows, :],
                                      ot[:rows, :])

    nc.compile()
    return nc


def kernel(**inputs):
    from concourse.bass_utils import run_bass_kernel_spmd

    cfg, in_maps = _host_prep(**inputs)
    nc = _build_nc(cfg)
    res = run_bass_kernel_spmd(nc, in_maps, core_ids=list(range(NCORE)))
    return np.concatenate([res.results[k]["out"] for k in range(NCORE)], axis=0)
